# revision 3
# baseline (speedup 1.0000x reference)
"""TRN2 Bass kernel for BasicLSTM (B=32, T=512, IN=512, H=1024). V2.

Tensor-parallel over the 4H gate dim across 8 cores (512 gate cols per
core, order [i|f|o|g] so i,f,o are contiguous for one merged sigmoid).

V2 vs baseline:
  - Phase A (x@W+b) interleaved into the recurrence: one 128-row tile
    every 4 steps, emitted into the AllGather window; xz lives in an
    SBUF ring (no DRAM xzb round-trip, no per-step xzt prefetch).
  - One sigmoid over [i|f|o] (384 cols) instead of two activations.
  - h history buffered 4 steps; hs output DMA'd every 4th step.
  - cc_in write issued from the DVE queue right after the transpose.
"""

import numpy as np

import concourse.bass as bass
import concourse.mybir as mybir
import concourse.tile as tile
from concourse import bacc, bass_utils
from concourse.bass import ts, ds
from concourse.masks import make_identity

B = 32
T = 512
IN = 512
H = 1024
NCORES = 8
NS = 4 * H // NCORES  # 512 gate cols per core
HS = H // NCORES      # 128 h cols per core
F32 = mybir.dt.float32
F32R = mybir.dt.float32r
AF = mybir.ActivationFunctionType
LOOK = 2   # phase-A tiles of lookahead (8 steps)
RING = 4   # xz ring slots


def _build(t_steps: int = T, ablate: frozenset = frozenset(), reps: int = 1):
    """ablate (perf experiments only, breaks numerics):
    'noag' - skip the AllGather (stale hT);  'nomm' - skip recurrent matmuls."""
    assert t_steps % 4 == 0
    ntiles = t_steps * B // 128  # phase-A tiles (4 timesteps each)
    nc = bacc.Bacc("TRN2", debug=False, num_devices=NCORES)

    x_d = nc.dram_tensor("x", [B, t_steps, IN], F32, kind="ExternalInput")
    w_d = nc.dram_tensor("w", [IN, NS], F32, kind="ExternalInput")
    u_d = nc.dram_tensor("u", [H, NS], F32, kind="ExternalInput")
    b_d = nc.dram_tensor("b", [1, NS], F32, kind="ExternalInput")
    hs_d = nc.dram_tensor("hs", [B, t_steps, HS], F32, kind="ExternalOutput")
    cc_in = [nc.dram_tensor(f"cc_in{i}", [HS, B], F32R) for i in range(2)]
    cc_out = [
        nc.dram_tensor(f"cc_out{i}", [H, B], F32R, addr_space="Shared")
        for i in range(2)
    ]
    RG = [list(range(NCORES))]

    # Long-lived constants/weights/state: static SBUF allocations.
    id128 = nc.alloc_sbuf_tensor("id128", [128, 128], F32).ap()
    id32q = nc.alloc_sbuf_tensor("id32q", [128, 32], F32R).ap()
    ones1 = nc.alloc_sbuf_tensor("ones1", [1, 128], F32R).ap()
    b_sb = nc.alloc_sbuf_tensor("b_sb", [1, NS], F32R).ap()
    wk = [nc.alloc_sbuf_tensor(f"wk{j}", [128, NS], F32R).ap() for j in range(IN // 128)]
    uk = [nc.alloc_sbuf_tensor(f"uk{j}", [128, NS], F32R).ap() for j in range(H // 128)]
    c_bufs = [nc.alloc_sbuf_tensor(f"c_st{i}", [B, HS], F32).ap() for i in range(2)]
    xzr = [nc.alloc_sbuf_tensor(f"xzr{i}", [128, NS], F32R).ap() for i in range(RING)]
    # matmul operands must sit at base partition 0/32/64 — the 4th timestep
    # quarter (base 96) gets its own copy at base 0.
    xz3 = [nc.alloc_sbuf_tensor(f"xz3_{i}", [32, NS], F32R).ap() for i in range(RING)]
    h_hist = nc.alloc_sbuf_tensor("h_hist", [B, 4 * HS], F32).ap()

    with tile.TileContext(nc) as tc:
        with (
            tc.tile_pool(name="xin", bufs=3) as xin_pool,
            tc.tile_pool(name="xtr", bufs=4) as xt_pool,
            tc.tile_pool(name="psA", bufs=2, space=bass.MemorySpace.PSUM) as psA_pool,
            tc.tile_pool(name="psT", bufs=2, space=bass.MemorySpace.PSUM) as psT_pool,
            tc.tile_pool(name="gates", bufs=2) as g_pool,
            tc.tile_pool(name="hT", bufs=2) as hT_pool,
            tc.tile_pool(name="psB", bufs=2, space=bass.MemorySpace.PSUM) as psB_pool,
        ):
            nc.any.memset(c_bufs[0], 0.0)
            make_identity(nc, id128)
            id32_dram = nc.inline_tensor(
                np.tile(np.eye(32, dtype=np.float32), (4, 1)), name="id32c"
            )
            nc.gpsimd.dma_start(id32q, id32_dram.ap())
            ones_dram = nc.inline_tensor(np.ones((1, 128), np.float32), name="ones1c")
            nc.gpsimd.dma_start(ones1, ones_dram.ap())
            nc.gpsimd.dma_start(b_sb, b_d.ap())
            for j in range(IN // 128):
                nc.gpsimd.dma_start(wk[j], w_d.ap()[ts(j, 128), :])
            for j in range(H // 128):
                nc.gpsimd.dma_start(uk[j], u_d.ap()[ts(j, 128), :])

            def phase_a_tile(m: int):
                """xz tile m (timesteps 4m..4m+3) -> xzr[m % RING]."""
                t0 = m * 4
                xt_in = xin_pool.tile([128, IN], F32, tag="xin")
                nc.sync.dma_start(
                    xt_in[:, :],
                    x_d.ap()[:, ds(t0, 4), :].rearrange("b t i -> t b i"),
                )
                zp = psA_pool.tile([128, NS], F32, tag="zpa")
                for j in range(IN // 128):
                    xTp = psT_pool.tile([128, 128], F32, tag="xTp")
                    nc.tensor.transpose(xTp, xt_in[:, ts(j, 128)], id128)
                    xTs = xt_pool.tile([128, 128], F32R, tag="xTs")
                    nc.vector.tensor_copy(xTs, xTp)
                    nc.tensor.matmul(zp, xTs, wk[j], start=(j == 0), stop=False)
                nc.tensor.matmul(zp, ones1, b_sb, start=False, stop=True)
                nc.vector.tensor_copy(xzr[m % RING], zp)
                nc.gpsimd.dma_start(xz3[m % RING], xzr[m % RING][ds(96, 32), :])

            for _rep in range(reps):
                for m in range(min(LOOK, ntiles)):
                    phase_a_tile(m)

                hT_cur = None  # h_0 == 0 -> step 0 skips the recurrent matmuls
                for t in range(t_steps):
                    if t % 4 == 3:
                        xz_t = xz3[(t // 4) % RING]
                        id_t = id32q[ds(0, 32), :]
                    else:
                        xz_t = xzr[(t // 4) % RING][ds((t % 4) * 32, 32), :]
                        id_t = id32q[ds((t % 4) * 32, 32), :]
                    zp = psB_pool.tile([B, NS], F32, tag="zpb")
                    if t == 0 or "nomm" in ablate:
                        nc.tensor.matmul(zp, id_t, xz_t, start=True, stop=True)
                    else:
                        # xz seed first: it only needs the SBUF ring, so it
                        # runs during the previous step's AllGather window.
                        nc.tensor.matmul(zp, id_t, xz_t, start=True, stop=False)
                        # chunks 0-3 read hT_a (first half of the gather),
                        # 4-7 read hT_b - so they start as halves land.
                        for j in range(H // 128):
                            src = hT_cur[j // 4]
                            nc.tensor.matmul(
                                zp, src[:, ts(j % 4, 32)], uk[j],
                                start=False, stop=(j == H // 128 - 1),
                            )

                    # gate columns: [i | f | o | g]; sigmoid split so fc can
                    # start early and sig_o fills the Act gap.
                    sif = g_pool.tile([B, 2 * HS], F32, tag="sif")
                    nc.scalar.activation(sif, zp[:, 0:2 * HS], AF.Sigmoid)
                    g_t = g_pool.tile([B, HS], F32, tag="g")
                    nc.scalar.activation(g_t, zp[:, 3 * HS:4 * HS], AF.Tanh)
                    so = g_pool.tile([B, HS], F32, tag="so")
                    nc.scalar.activation(so, zp[:, 2 * HS:3 * HS], AF.Sigmoid)

                    fc = g_pool.tile([B, HS], F32, tag="fc")
                    nc.vector.tensor_mul(fc, sif[:, HS:2 * HS], c_bufs[t % 2])
                    ig = g_pool.tile([B, HS], F32, tag="ig")
                    nc.vector.tensor_mul(ig, sif[:, 0:HS], g_t)
                    c_new = c_bufs[(t + 1) % 2]
                    nc.vector.tensor_add(c_new, ig, fc)

                    tc_t = g_pool.tile([B, HS], F32, tag="tc")
                    nc.scalar.activation(tc_t, c_new, AF.Tanh)
                    h_t = h_hist[:, ds((t % 4) * HS, HS)]
                    nc.vector.tensor_mul(h_t, so, tc_t)

                    if t % 4 == 3 or t == t_steps - 1:
                        tlo = (t // 4) * 4
                        nn = t - tlo + 1
                        nc.sync.dma_start(
                            hs_d.ap()[:, ds(tlo, nn), :],
                            h_hist[:, 0:nn * HS].rearrange("b (t h) -> b t h", t=nn),
                        )

                    if t < t_steps - 1:
                        # h^T shard -> DRAM -> AllGather -> SBUF for next step.
                        # vector.transpose is 32x32-block-wise; the DMA's
                        # rearrange scatters the 4 blocks into h^T rows.
                        htr = g_pool.tile([B, HS], F32, tag="htr")
                        nc.vector.transpose(htr, h_t)
                        buf = t % 2
                        nc.scalar.dma_start(
                            cc_in[buf].ap().bitcast(F32)
                            .rearrange("(j p) q -> p j q", j=4),
                            htr.rearrange("p (j q) -> p j q", j=4),
                        )
                        if "noag" in ablate:
                            hT_a = hT_pool.tile([128, 4 * B], F32R, tag="hTa")
                            hT_b = hT_pool.tile([128, 4 * B], F32R, tag="hTb")
                            nc.sync.dma_start(hT_a[:, 0:B], cc_in[buf].ap())
                            nc.sync.dma_start(hT_b[:, 0:B], cc_in[buf].ap())
                            hT_cur = (hT_a, hT_b)
                        else:
                            nc.gpsimd.collective_compute(
                                "AllGather",
                                mybir.AluOpType.bypass,
                                replica_groups=RG,
                                ins=[cc_in[buf].ap().opt()],
                                outs=[cc_out[buf].ap().opt()],
                            )
                            # two half-gather reads into separate tiles so
                            # chunk matmuls 0-3 start while 4-7's data lands
                            hT_a = hT_pool.tile([128, 4 * B], F32R, tag="hTa")
                            hT_b = hT_pool.tile([128, 4 * B], F32R, tag="hTb")
                            nc.sync.dma_start(
                                hT_a.rearrange("p (j b) -> p j b", j=4),
                                cc_out[buf].ap()[ds(0, H // 2), :]
                                .rearrange("(j p) b -> p j b", p=128),
                            )
                            nc.sync.dma_start(
                                hT_b.rearrange("p (j b) -> p j b", j=4),
                                cc_out[buf].ap()[ds(H // 2, H // 2), :]
                                .rearrange("(j p) b -> p j b", p=128),
                            )
                            hT_cur = (hT_a, hT_b)

                    # phase-A tile for 8 steps ahead, into the AG window
                    if t % 4 == 0 and t // 4 + LOOK < ntiles:
                        phase_a_tile(t // 4 + LOOK)

    nc.compile()
    return nc


def _make_in_maps(x, W, U, b, t_steps: int = T):
    x = np.asarray(x, np.float32)[:, :t_steps, :]
    W = np.asarray(W, np.float32)
    U = np.asarray(U, np.float32)
    b = np.asarray(b, np.float32)
    in_maps = []
    for k in range(NCORES):
        # per-core gate column order: [i | f | o | g]
        cols = np.concatenate(
            [np.arange(k * HS, (k + 1) * HS) + gofs * H for gofs in (0, 1, 3, 2)]
        )
        in_maps.append(
            {
                "x": np.ascontiguousarray(x),
                "w": np.ascontiguousarray(W[:, cols]),
                "u": np.ascontiguousarray(U[:, cols]),
                "b": np.ascontiguousarray(b[cols]).reshape(1, NS),
            }
        )
    return in_maps


def _pjrt_bundle(nc, n_reps: int = 1):
    """Reusable sharded PJRT executable (see kernel.py baseline for notes)."""
    import jax
    from jax.experimental.shard_map import shard_map
    from jax.sharding import Mesh, PartitionSpec
    from concourse import bass2jax

    bass2jax.install_neuronx_cc_hook()
    partition_name = nc.partition_id_tensor.name if nc.partition_id_tensor else None
    in_names, out_names, out_avals, zero_outs = [], [], [], []
    for alloc in nc.m.functions[0].allocations:
        if not isinstance(alloc, mybir.MemoryLocationSet):
            continue
        name = alloc.memorylocations[0].name
        if alloc.kind == "ExternalInput":
            if name != partition_name:
                in_names.append(name)
        elif alloc.kind == "ExternalOutput":
            shape = tuple(alloc.tensor_shape)
            dtype = mybir.dt.np(alloc.dtype)
            out_names.append(name)
            out_avals.append(jax.core.ShapedArray(shape, dtype))
            zero_outs.append(np.zeros(shape, dtype))
    n_params = len(in_names)
    n_outs = len(out_avals)
    all_in_names = list(in_names) + list(out_names)
    if partition_name is not None:
        all_in_names.append(partition_name)

    def _body(*args):
        ins = list(args[:n_params])
        zs = list(args[n_params:])
        for _ in range(n_reps):
            operands = ins + zs
            if partition_name is not None:
                operands.append(bass2jax.partition_id_tensor())
            outs = bass2jax._bass_exec_p.bind(
                *operands,
                out_avals=tuple(out_avals),
                in_names=tuple(all_in_names),
                out_names=tuple(out_names),
                lowering_input_output_aliases=(),
                sim_require_finite=True,
                sim_require_nnan=True,
                nc=nc,
            )
            zs = list(outs)
        return tuple(outs)

    devices = jax.devices()[:NCORES]
    mesh = Mesh(np.asarray(devices), ("core",))
    in_specs = (PartitionSpec("core"),) * (n_params + n_outs)
    out_specs = (PartitionSpec("core"),) * n_outs
    sharded = jax.jit(
        shard_map(
            _body, mesh=mesh, in_specs=in_specs, out_specs=out_specs, check_rep=False
        ),
        donate_argnums=tuple(range(n_params, n_params + n_outs)),
        keep_unused=True,
    )
    return dict(
        fn=sharded,
        mesh=mesh,
        in_names=in_names,
        out_names=out_names,
        out_avals=out_avals,
        zero_outs=zero_outs,
        n_params=n_params,
    )


def _run(inputs, t_steps: int = T, trace: bool = False):
    nc = _build(t_steps)
    in_maps = _make_in_maps(inputs["x"], inputs["W"], inputs["U"], inputs["b"], t_steps)
    res = bass_utils.run_bass_kernel_spmd(
        nc, in_maps, core_ids=list(range(NCORES)), trace=trace
    )
    out = np.empty((B, t_steps, H), np.float32)
    for k in range(NCORES):
        out[:, :, k * HS:(k + 1) * HS] = res.results[k]["hs"]
    return out, res


def kernel(**inputs) -> np.ndarray:
    out, _ = _run(inputs)
    return out
